# revision 1
# baseline (speedup 1.0000x reference)
"""Trainium2 Bass kernel for a contrastive hinge loss.

Problem (B=32 splits, L=1024 candidates/split, P=8 positives/split, D=256):
    e = l2norm(sent), q = l2norm(query)
    sim[b,l] = e[b,l] . q[b]
    loss = sum_{b, p in pos_b, j in neg_b} relu(sim[b,j] - sim[b,p] + margin) / total
    total = sum_b |pos_b| * |neg_b|

Strategy (data-parallel over B across 8 cores, 4 splits per core):
  Device (per core), all heavy math on-chip:
    - x tile per split in natural [128 part, 8*256] layout (partition p holds
      candidates l = 8p..8p+7, fully contiguous 8KB DMA per partition),
      loads alternating over the two HWDGE rings so they run in parallel.
    - qhat replicated to all partitions via a K=1 matmul on the PE.
    - dot[l] = sum_d x[l,d] * qhat[b,d]  via fused DVE scalar_tensor_tensor
    - ssq[l] = sum_d x[l,d]^2            via ACT Square + accum (some on DVE)
    - sim = dot / sqrt(ssq)              (fused DVE divide)
    - s_vec[b,j] = sim at positive j     via one-hot matmuls on the PE
    - G[b,j] = sum_{l in ALL} relu(sim[b,l] - s_vec[b,j] + margin)
      via broadcasted add + relu (GpSimd) + PE column-sum, per split so the
      tail pipelines under the next split's heavy passes.
  Host:
    - normalizes queries (32x256, trivial), builds the one-hot PH from pos_idx,
      and finishes: loss = [sum G[b,j over unique positives]
                            - sum_{p,q in pos_b} relu(s_q - s_p + margin)] / total
      using the device-returned s_vec (exact cancellation of pos-as-neg terms).

Handles duplicate pos_idx entries (dedup on host; G is per-(b,j) so duplicate
columns are simply not counted twice).
"""

import numpy as np

B, L, P, D = 32, 1024, 8, 256
NCORES = 8
BL = B // NCORES          # 4 splits per core
U = L // 128              # 8 candidates per partition
MARGIN = 0.01

_CACHED = {}


def _build_nc():
    import concourse.bass as bass
    import concourse.mybir as mybir
    import concourse.tile as tile
    from concourse import bacc

    f32 = mybir.dt.float32
    Alu = mybir.AluOpType
    Act = mybir.ActivationFunctionType

    # Bacc (not raw Bass): its compile() runs generate_event_semaphores, which
    # splits multi-wait instructions — walrus allows 1 sync wait per op.
    nc = bacc.Bacc("TRN2")
    # x[b, p, u*D + d] = sent[core*BL + b, 8p + u, d]  (pure reshape on host)
    x = nc.dram_tensor("x", [BL, 128, U * D], f32, kind="ExternalInput")
    # host-normalized queries, concatenated as one row
    qh = nc.dram_tensor("qh", [1, BL * D], f32, kind="ExternalInput")
    # one-hot: ph[p, b, u, j] = 1 if pos_idx[b][j] == 8p + u else 0
    ph = nc.dram_tensor("ph", [128, BL, U, P], f32, kind="ExternalInput")
    # out[0, 0:32]  = G[b, j]   (sum over ALL candidates of relu(sim - s_bj + m))
    # out[0, 32:64] = s_vec[b, j]
    out = nc.dram_tensor("out", [1, BL * P * 2], f32, kind="ExternalOutput")

    C = BL * U  # 32 (b, u) columns
    with tile.TileContext(nc) as tc:
        with (
            tc.tile_pool(name="singles", bufs=1) as singles,
            tc.tile_pool(name="xpool", bufs=4) as xpool,
            tc.tile_pool(name="pp", bufs=1, space="PSUM") as pp,
            tc.tile_pool(name="pstail", bufs=2, space="PSUM") as pstail,
        ):
            # ---- loads up-front, spread over the three DMA rings ----
            # x loads alternate between the two HWDGE rings (SP + ACT) so they
            # run in parallel; qrep/ph go via SWDGE (gpsimd).
            xts = []
            for b in range(BL):
                xt = xpool.tile([128, U * D], f32, tag=f"xt{b}")
                eng = nc.sync if b % 2 == 0 else nc.scalar
                eng.dma_start(out=xt[:, :], in_=x[b, :, :])
                xts.append(xt)

            qh_sb = singles.tile([1, BL * D], f32)
            nc.gpsimd.dma_start(out=qh_sb[:, :], in_=qh[:, :])
            ph_sb = singles.tile([128, BL, U, P], f32)
            nc.gpsimd.dma_start(out=ph_sb[:, :, :, :], in_=ph[:, :, :, :])

            ones_row = singles.tile([1, 128], f32)  # lhsT for partition-replication
            nc.vector.memset(ones_row[:, :], 1.0)
            ones_col = singles.tile([128, 1], f32)  # lhsT for partition-sum
            nc.vector.memset(ones_col[:, :], 1.0)

            # Warm the ACT table set covering Square (whose set also holds
            # Sqrt/Relu/Copy) so only ONE PSEUDO_LOAD_ACT_FUNC_SET happens.
            warm = singles.tile([1, 1], f32)
            nc.vector.memset(warm[:, :], 1.0)
            nc.scalar.activation(out=warm[0:1, :], in_=warm[0:1, :],
                                 func=Act.Square)

            # replicate qhat rows to all 128 partitions on the PE
            psum_qrep = pp.tile([128, BL * D], f32)
            for h in range(2):
                nc.tensor.matmul(
                    psum_qrep[:, h * 512:(h + 1) * 512],
                    lhsT=ones_row[0:1, :],
                    rhs=qh_sb[0:1, h * 512:(h + 1) * 512],
                    start=True, stop=True)
            qrep = singles.tile([128, BL, D], f32)
            nc.vector.tensor_copy(
                out=qrep[:, :, :],
                in_=psum_qrep[:, :].rearrange("p (b d) -> p b d", b=BL))

            dot_all = singles.tile([128, C], f32)
            ssq_all = singles.tile([128, C], f32)
            sim_all = singles.tile([128, C], f32)
            # two junk buffers per engine: consecutive same-engine ops then
            # WAW-depend two-back, which Tile sees as already-observed -> no
            # extra event-semaphore per op.
            junk_dve = [singles.tile([128, D], f32, name=f"jd{i}", tag=f"jd{i}")
                        for i in range(2)]
            junk_act = [singles.tile([128, D], f32, name=f"ja{i}", tag=f"ja{i}")
                        for i in range(2)]
            gs = singles.tile([1, BL * P * 2], f32)
            ms_row = singles.tile([1, BL * P], f32)
            msrep = singles.tile([128, BL * P], f32)
            pair = singles.tile([128, BL, U * P], f32)
            rel = singles.tile([128, BL, U * P], f32)

            for b in range(BL):
                xt = xts[b]
                # ---- heavy passes ----
                for u in range(U):
                    c = b * U + u
                    xs = xt[:, u * D:(u + 1) * D]
                    nc.vector.scalar_tensor_tensor(
                        out=junk_dve[u % 2][:, :],
                        in0=xs,
                        scalar=1.0,
                        in1=qrep[:, b, :],
                        op0=Alu.mult,
                        op1=Alu.mult,
                        accum_out=dot_all[:, c:c + 1],
                    )
                    if c % 5 != 4:
                        nc.scalar.activation(
                            out=junk_act[u % 2][:, :],
                            in_=xs,
                            func=Act.Square,
                            accum_out=ssq_all[:, c:c + 1],
                        )
                    else:
                        nc.vector.scalar_tensor_tensor(
                            out=junk_dve[u % 2][:, :],
                            in0=xs,
                            scalar=1.0,
                            in1=xs,
                            op0=Alu.mult,
                            op1=Alu.mult,
                            accum_out=ssq_all[:, c:c + 1],
                        )
                bsl = slice(b * U, (b + 1) * U)
                # sim = dot * 1/sqrt(ssq)
                nc.scalar.activation(
                    out=ssq_all[:, bsl], in_=ssq_all[:, bsl], func=Act.Sqrt)
                nc.vector.reciprocal(out=ssq_all[:, bsl], in_=ssq_all[:, bsl])
                nc.vector.tensor_mul(
                    out=sim_all[:, bsl], in0=dot_all[:, bsl], in1=ssq_all[:, bsl])

                # ---- per-split tail (pipelines under split b+1) ----
                psum_s = pstail.tile([1, P], f32, tag="ps_s")
                for u in range(U):
                    c = b * U + u
                    nc.tensor.matmul(
                        psum_s[0:1, :],
                        lhsT=sim_all[:, c:c + 1],
                        rhs=ph_sb[:, b, u, :],
                        start=(u == 0),
                        stop=(u == U - 1),
                    )
                # s_vec -> host output; ms = margin - s_vec
                nc.vector.tensor_copy(
                    out=gs[0:1, BL * P + b * P:BL * P + (b + 1) * P],
                    in_=psum_s[0:1, :])
                nc.scalar.activation(
                    out=ms_row[0:1, b * P:(b + 1) * P], in_=psum_s[0:1, :],
                    func=Act.Copy, bias=float(MARGIN), scale=-1.0)
                psum_msrep = pstail.tile([128, P], f32, tag="ps_m", bufs=1)
                nc.tensor.matmul(
                    psum_msrep[:, :], lhsT=ones_row[0:1, :],
                    rhs=ms_row[0:1, b * P:(b + 1) * P], start=True, stop=True)
                nc.vector.tensor_copy(
                    out=msrep[:, b * P:(b + 1) * P], in_=psum_msrep[:, :])
                # pair[p, u, j] = sim[p, (b,u)] + ms[b, j]; relu via max(,0)
                sim_b = (sim_all[:, bsl]
                         .unsqueeze(2)
                         .broadcast_to((128, U, P)))
                ms_b = (msrep[:, b * P:(b + 1) * P]
                        .unsqueeze(1)
                        .broadcast_to((128, U, P)))
                nc.gpsimd.tensor_add(
                    out=pair[:, b, :].rearrange("p (u j) -> p u j", u=U),
                    in0=sim_b, in1=ms_b)
                nc.vector.tensor_scalar_max(
                    out=rel[:, b, :], in0=pair[:, b, :], scalar1=0.0)
                psum_g = pstail.tile([1, U * P], f32, tag="ps_g")
                nc.tensor.matmul(
                    psum_g[0:1, :], lhsT=ones_col[:, 0:1], rhs=rel[:, b, :],
                    start=True, stop=True)
                nc.vector.tensor_reduce(
                    out=gs[0:1, b * P:(b + 1) * P]
                        .rearrange("p (o j) -> p o j", o=1),
                    in_=psum_g[0:1, :].rearrange("p (u j) -> p j u", u=U),
                    axis=mybir.AxisListType.X,
                    op=Alu.add,
                )

            nc.sync.dma_start(out=out[0:1, :], in_=gs[0:1, :])

    nc.finalize()
    return nc


def _get_nc():
    if "nc" not in _CACHED:
        _CACHED["nc"] = _build_nc()
    return _CACHED["nc"]


def _host_prep(sent, query, pos_idx):
    """Build per-core input maps."""
    sent = np.ascontiguousarray(sent, dtype=np.float32)
    query = np.asarray(query, dtype=np.float32)
    pos_idx = np.asarray(pos_idx).astype(np.int64)

    qn = np.linalg.norm(query, axis=-1, keepdims=True)
    qhat = (query / np.maximum(qn, 1e-12)).astype(np.float32)

    ph = np.zeros((B, 128, U, P), dtype=np.float32)
    bb = np.repeat(np.arange(B), P)
    ll = pos_idx.reshape(-1)
    jj = np.tile(np.arange(P), B)
    ph[bb, ll // U, ll % U, jj] = 1.0

    in_maps = []
    for core in range(NCORES):
        sl = slice(core * BL, (core + 1) * BL)
        in_maps.append({
            "x": sent[sl].reshape(BL, 128, U * D),
            "qh": qhat[sl].reshape(1, BL * D),
            "ph": np.ascontiguousarray(ph[sl].transpose(1, 0, 2, 3)),
        })
    return in_maps, pos_idx


def _host_finish(results, pos_idx):
    """Combine per-core (G[b,j], s_vec[b,j]) into the scalar loss."""
    g = np.zeros((B, P), dtype=np.float64)
    s = np.zeros((B, P), dtype=np.float64)
    for core, res in enumerate(results):
        o = res["out"].reshape(2, B // NCORES, P)
        g[core * BL:(core + 1) * BL] = o[0]
        s[core * BL:(core + 1) * BL] = o[1]

    loss = 0.0
    total = 0
    for b in range(B):
        _, first = np.unique(pos_idx[b], return_index=True)
        npos = len(first)
        total += npos * (L - npos)
        sb = s[b, first]
        loss += g[b, first].sum()
        loss -= np.maximum(sb[None, :] - sb[:, None] + MARGIN, 0.0).sum()
    return np.float32(loss / total)


def kernel(sent_embeddings, query_embeddings, pos_idx, splits=None, **_):
    import sys
    if "/opt/trn_rl_repo" not in sys.path:
        sys.path.insert(0, "/opt/trn_rl_repo")
    from concourse.bass_utils import run_bass_kernel_spmd

    in_maps, pos_idx = _host_prep(sent_embeddings, query_embeddings, pos_idx)
    nc = _get_nc()
    res = run_bass_kernel_spmd(nc, in_maps, core_ids=list(range(NCORES)))
    _CACHED["last_result"] = res
    return _host_finish(res.results, pos_idx)


if __name__ == "__main__":
    rng = np.random.default_rng(0)
    sent = rng.standard_normal((B, L, D), dtype=np.float32)
    query = rng.standard_normal((B, D), dtype=np.float32)
    pidx = np.stack([rng.choice(L, P, replace=False) for _ in range(B)])
    print(kernel(sent, query, pidx, L))



# revision 5
# speedup vs baseline: 1.2428x; 1.2428x over previous
"""Trainium2 Bass kernel for a contrastive hinge loss.

Problem (B=32 splits, L=1024 candidates/split, P=8 positives/split, D=256):
    e = l2norm(sent), q = l2norm(query)
    sim[b,l] = e[b,l] . q[b]
    loss = sum_{b, p in pos_b, j in neg_b} relu(sim[b,j] - sim[b,p] + margin) / total

Strategy (data-parallel over B across 8 cores, 4 splits per core), v2:
  Inputs are downcast to bf16 on the host (tolerance is 2e-2; bf16 keeps the
  loss within ~1e-3) which halves HBM traffic -- the roofline for this
  memory-regime problem -- and x is sent pre-transposed to d-major layout
  [d, l] so the 256-long dot reductions run on the PE (128 MACs/col/cycle)
  instead of element-serial DVE/ACT accumulation:
    - dot rows:  prd[b, l] = sum_d q[b,d] x[b,d,l]   (PE, q as 4-col weights)
    - ssq rows:  prq[b, l] = sum_d x[b,d,l]^2        (DVE square, PE ones-fold;
                                                      gpsimd pre-folds the two
                                                      d-halves for 2 splits)
    - sim = dot * rsqrt(ssq) on [4, 512] rows (ACT sqrt, DVE approx-recip, mult)
    - one tiny PE matmul broadcasts sim rows to 32 (b,j) partitions, then a
      SINGLE ACT Relu with per-partition bias (margin - s_bj) and accumulator
      computes G[b,j] = sum_l relu(sim_l - s_bj + margin) per chunk.
    - s_vec[b,j] comes from host-gathered positive embeddings xpos [32, 256]
      (candidate-major, partition = (b,j)): STT dot + ACT square-accum +
      sqrt/recip/mult. This replaces the one-hot matmul gather entirely.
  Host: normalizes queries, gathers xpos, packs weights; finishes with the
  same dedup + pos-pos hinge correction as before:
    loss = [sum G[b,j] - sum_{p,q in pos_b} relu(s_q - s_p + margin)] / total
"""

import numpy as np

B, L, P, D = 32, 1024, 8, 256
NCORES = 8
BL = B // NCORES          # 4 splits per core
H = 2                     # d-halves (256 = 2*128)
C = 2                     # l-chunks (1024 = 2*512 psum-bank limit)
CH = 512
MARGIN = 0.01

_CACHED = {}


def _build_nc():
    import concourse.bass as bass
    import concourse.mybir as mybir
    import concourse.tile as tile
    from concourse import bacc

    f32 = mybir.dt.float32
    b16 = mybir.dt.bfloat16
    Alu = mybir.AluOpType
    Act = mybir.ActivationFunctionType

    nc = bacc.Bacc("TRN2")
    # x[b, p, h*1024 + l] = sent_T[b, 128h + p, l]  (host-transposed, bf16)
    x = nc.dram_tensor("x", [BL, 128, H * L], b16, kind="ExternalInput")
    # weight cols: block 4*(2b+h) has qhat[b, 128h:128h+128] at local col b;
    # block 32+4b has ones at local col b (ssq fold); cols 48:80 rows 0-3 are
    # the simrep indicator ind[b, 8b+j] = 1.
    wts = nc.dram_tensor("wts", [128, 80], b16, kind="ExternalInput")
    # aux[8b+j, 0:256] = sent[b, pos_idx[b,j], :]; aux[., 256:512] = qhat[b]
    aux = nc.dram_tensor("aux", [BL * P, 2 * D], b16, kind="ExternalInput")
    # out[8b+j] = (G[b,j], s_vec[b,j])
    out = nc.dram_tensor("out", [BL * P, 2], f32, kind="ExternalOutput")

    with tile.TileContext(nc) as tc:
        with (
            tc.tile_pool(name="singles", bufs=1) as singles,
            tc.tile_pool(name="xpool", bufs=4) as xpool,
            tc.tile_pool(name="sqpool", bufs=2) as sqpool,
            tc.tile_pool(name="pp", bufs=1, space="PSUM") as pp,
        ):
            # ---- ACT table warm: Sqrt forces the sqrt_and_others set, which
            # also holds Square/Relu/Copy -- one table load total.
            warm = singles.tile([1, 1], f32)
            nc.vector.memset(warm[:, :], 1.0)
            nc.scalar.activation(out=warm[0:1, :], in_=warm[0:1, :],
                                 func=Act.Sqrt)

            # ---- small loads on the SWDGE ring (parallel to x loads) ----
            wts_sb = singles.tile([128, 80], b16)
            nc.gpsimd.dma_start(out=wts_sb[:, :], in_=wts[:, :])
            aux_sb = singles.tile([BL * P, 2 * D], b16)
            nc.gpsimd.dma_start(out=aux_sb[:, :], in_=aux[:, :])

            # ---- x loads, one per split, on the SP HWDGE ring ----
            xts = []
            for b in range(BL):
                xt = xpool.tile([128, H * L], b16, tag=f"xt{b}")
                nc.sync.dma_start(out=xt[:, :], in_=x[b, :, :])
                xts.append(xt)

            # ---- s_vec from host-gathered positives (under the x DMAs) ----
            dp = singles.tile([BL * P, 1], f32)
            sp = singles.tile([BL * P, 1], f32)
            junka = singles.tile([BL * P, D], b16)
            junkb = singles.tile([BL * P, D], b16)
            rtp = singles.tile([BL * P, 1], f32)
            svec = singles.tile([BL * P, 1], f32)
            ms = singles.tile([BL * P, 1], f32)
            outsb = singles.tile([BL * P, 2], f32)
            nc.vector.scalar_tensor_tensor(
                out=junka[:, :], in0=aux_sb[:, 0:D], scalar=1.0,
                in1=aux_sb[:, D:2 * D], op0=Alu.mult, op1=Alu.mult,
                accum_out=dp[:, :])
            nc.scalar.activation(out=junkb[:, :], in_=aux_sb[:, 0:D],
                                 func=Act.Square, accum_out=sp[:, :])
            nc.scalar.activation(out=rtp[:, :], in_=sp[:, :], func=Act.Sqrt)
            nc.vector.reciprocal(out=rtp[:, :], in_=rtp[:, :])
            nc.vector.tensor_mul(out=svec[:, :], in0=dp[:, :], in1=rtp[:, :])
            nc.vector.tensor_copy(out=outsb[:, 1:2], in_=svec[:, :])
            # ms = margin - s
            nc.scalar.activation(out=ms[:, :], in_=svec[:, :], func=Act.Copy,
                                 bias=float(MARGIN), scale=-1.0)

            # ---- heavy pass: dot rows prd[c], ssq rows prq[c] (psum) ----
            prd = [pp.tile([BL, CH], f32, name=f"prd{c}", tag=f"prd{c}")
                   for c in range(C)]
            prq = [pp.tile([BL, CH], f32, name=f"prq{c}", tag=f"prq{c}")
                   for c in range(C)]
            pss = [pp.tile([BL * P, CH], f32, name=f"ps{c}", tag=f"ps{c}")
                   for c in range(C)]
            sqfs = [singles.tile([128, L], b16, name=f"sqf{b}", tag=f"sqf{b}")
                    for b in range(2)]

            nd = [0, 0]                   # dot MMs emitted per chunk
            nq = [0, 0]                   # fold MMs emitted per chunk
            ND_TOTAL = BL * H             # 8 dot MMs per chunk
            NQ_TOTAL = 2 * 1 + 2 * H      # 6 fold MMs per chunk

            for b in range(BL):
                xt = xts[b]
                # dot: prd[c][b, :] += qhat[b,h] . x[b,h]
                for c in range(C):
                    for h in range(H):
                        nc.tensor.matmul(
                            prd[c][0:BL, :],
                            lhsT=wts_sb[:, 4 * (2 * b + h):4 * (2 * b + h) + 4],
                            rhs=xt[:, h * L + c * CH:h * L + c * CH + CH],
                            start=(nd[c] == 0),
                            stop=(nd[c] == ND_TOTAL - 1),
                        )
                        nd[c] += 1
                # squares (one big bf16 DVE op per split)
                sq = sqpool.tile([128, H * L], b16, tag="sq")
                nc.vector.tensor_mul(out=sq[:, :], in0=xt[:, :], in1=xt[:, :])
                if b < 2:
                    # pre-fold the two d-halves on gpsimd (otherwise idle)
                    nc.gpsimd.tensor_add(out=sqfs[b][:, :], in0=sq[:, 0:L],
                                         in1=sq[:, L:2 * L])
                    for c in range(C):
                        nc.tensor.matmul(
                            prq[c][0:BL, :],
                            lhsT=wts_sb[:, 32 + 4 * b:32 + 4 * b + 4],
                            rhs=sqfs[b][:, c * CH:c * CH + CH],
                            start=(nq[c] == 0),
                            stop=(nq[c] == NQ_TOTAL - 1),
                        )
                        nq[c] += 1
                else:
                    for c in range(C):
                        for h in range(H):
                            nc.tensor.matmul(
                                prq[c][0:BL, :],
                                lhsT=wts_sb[:, 32 + 4 * b:32 + 4 * b + 4],
                                rhs=sq[:, h * L + c * CH:h * L + c * CH + CH],
                                start=(nq[c] == 0),
                                stop=(nq[c] == NQ_TOTAL - 1),
                            )
                            nq[c] += 1

            # ---- tail per chunk: sim rows -> broadcast -> relu+accum ----
            rt = singles.tile([BL, C, CH], f32)
            sim = singles.tile([BL, C, CH], b16)
            junkr = [singles.tile([BL * P, CH], b16, name=f"jr{c}", tag=f"jr{c}")
                     for c in range(C)]
            gg = singles.tile([BL * P, C], f32)
            for c in range(C):
                nc.scalar.activation(out=rt[:, c, :], in_=prq[c][0:BL, :],
                                     func=Act.Sqrt)
                nc.vector.reciprocal_approx_fast(out=rt[:, c, :],
                                                 in_=rt[:, c, :])
                nc.vector.tensor_mul(out=sim[:, c, :], in0=prd[c][0:BL, :],
                                     in1=rt[:, c, :])
                nc.tensor.matmul(pss[c][:, :], lhsT=wts_sb[0:BL, 48:80],
                                 rhs=sim[:, c, :], start=True, stop=True)
                nc.scalar.activation(out=junkr[c][:, :], in_=pss[c][:, :],
                                     func=Act.Relu, bias=ms[:, :],
                                     accum_out=gg[:, c:c + 1])
            nc.vector.tensor_add(out=outsb[:, 0:1], in0=gg[:, 0:1],
                                 in1=gg[:, 1:2])

            nc.sync.dma_start(out=out[:, :], in_=outsb[:, :])

    nc.finalize()
    return nc


def _get_nc():
    if "nc" not in _CACHED:
        _CACHED["nc"] = _build_nc()
    return _CACHED["nc"]


def _host_prep(sent, query, pos_idx):
    """Build per-core input maps (bf16, d-major x, packed weights)."""
    import ml_dtypes

    bf16 = ml_dtypes.bfloat16
    sent = np.asarray(sent, dtype=np.float32)
    query = np.asarray(query, dtype=np.float32)
    pos_idx = np.asarray(pos_idx).astype(np.int64)

    qn = np.linalg.norm(query, axis=-1, keepdims=True)
    qhat = (query / np.maximum(qn, 1e-12)).astype(bf16)

    sent16 = sent.astype(bf16)
    # [B, 128, H*L]: xt[b, p, h*L + l] = sent[b, l, 128h + p]
    xt = np.ascontiguousarray(
        sent16.transpose(0, 2, 1)             # [B, D, L]
        .reshape(B, H, 128, L)                # [B, h, p, l]
        .transpose(0, 2, 1, 3)                # [B, p, h, l]
        .reshape(B, 128, H * L))

    in_maps = []
    for core in range(NCORES):
        sl = slice(core * BL, (core + 1) * BL)
        q = qhat[sl]                          # [BL, D]
        wts = np.zeros((128, 80), dtype=bf16)
        for b in range(BL):
            for h in range(H):
                wts[:, 4 * (2 * b + h) + b] = q[b, 128 * h:128 * h + 128]
            wts[:, 32 + 4 * b + b] = 1.0
            wts[b, 48 + 8 * b:48 + 8 * b + 8] = 1.0
        aux = np.zeros((BL * P, 2 * D), dtype=bf16)
        for b in range(BL):
            gb = core * BL + b
            aux[8 * b:8 * b + 8, 0:D] = sent16[gb, pos_idx[gb], :]
            aux[8 * b:8 * b + 8, D:2 * D] = q[b]
        in_maps.append({
            "x": xt[sl],
            "wts": wts,
            "aux": np.ascontiguousarray(aux),
        })
    return in_maps, pos_idx


def _host_finish(results, pos_idx):
    """Combine per-core (G[b,j], s_vec[b,j]) into the scalar loss."""
    g = np.zeros((B, P), dtype=np.float64)
    s = np.zeros((B, P), dtype=np.float64)
    for core, res in enumerate(results):
        o = res["out"].reshape(BL, P, 2)
        g[core * BL:(core + 1) * BL] = o[:, :, 0]
        s[core * BL:(core + 1) * BL] = o[:, :, 1]

    loss = 0.0
    total = 0
    for b in range(B):
        _, first = np.unique(pos_idx[b], return_index=True)
        npos = len(first)
        total += npos * (L - npos)
        sb = s[b, first]
        loss += g[b, first].sum()
        loss -= np.maximum(sb[None, :] - sb[:, None] + MARGIN, 0.0).sum()
    return np.float32(loss / total)


def kernel(sent_embeddings, query_embeddings, pos_idx, splits=None, **_):
    import sys
    if "/opt/trn_rl_repo" not in sys.path:
        sys.path.insert(0, "/opt/trn_rl_repo")
    from concourse.bass_utils import run_bass_kernel_spmd

    in_maps, pos_idx = _host_prep(sent_embeddings, query_embeddings, pos_idx)
    nc = _get_nc()
    res = run_bass_kernel_spmd(nc, in_maps, core_ids=list(range(NCORES)))
    _CACHED["last_result"] = res
    return _host_finish(res.results, pos_idx)


if __name__ == "__main__":
    rng = np.random.default_rng(0)
    sent = rng.standard_normal((B, L, D), dtype=np.float32)
    query = rng.standard_normal((B, D), dtype=np.float32)
    pidx = np.stack([rng.choice(L, P, replace=False) for _ in range(B)])
    print(kernel(sent, query, pidx, L))


# revision 6
# speedup vs baseline: 1.2948x; 1.0418x over previous
"""Trainium2 Bass kernel for a contrastive hinge loss.

Problem (B=32 splits, L=1024 candidates/split, P=8 positives/split, D=256):
    e = l2norm(sent), q = l2norm(query)
    sim[b,l] = e[b,l] . q[b]
    loss = sum_{b, p in pos_b, j in neg_b} relu(sim[b,j] - sim[b,p] + margin) / total

Strategy (data-parallel over B across 8 cores, 4 splits per core), v2:
  Inputs are downcast to bf16 on the host (tolerance is 2e-2; bf16 keeps the
  loss within ~1e-3) which halves HBM traffic -- the roofline for this
  memory-regime problem -- and x is sent pre-transposed to d-major layout
  [d, l] so the 256-long dot reductions run on the PE (128 MACs/col/cycle)
  instead of element-serial DVE/ACT accumulation:
    - dot rows:  prd[b, l] = sum_d q[b,d] x[b,d,l]   (PE, q as 4-col weights)
    - ssq rows:  prq[b, l] = sum_d x[b,d,l]^2        (DVE square, PE ones-fold;
                                                      gpsimd pre-folds the two
                                                      d-halves for 2 splits)
    - sim = dot * rsqrt(ssq) on [4, 512] rows (ACT sqrt, DVE approx-recip, mult)
    - one tiny PE matmul broadcasts sim rows to 32 (b,j) partitions, then a
      SINGLE ACT Relu with per-partition bias (margin - s_bj) and accumulator
      computes G[b,j] = sum_l relu(sim_l - s_bj + margin) per chunk.
    - s_vec[b,j] comes from host-gathered positive embeddings xpos [32, 256]
      (candidate-major, partition = (b,j)): STT dot + ACT square-accum +
      sqrt/recip/mult. This replaces the one-hot matmul gather entirely.
  Host: normalizes queries, gathers xpos, packs weights; finishes with the
  same dedup + pos-pos hinge correction as before:
    loss = [sum G[b,j] - sum_{p,q in pos_b} relu(s_q - s_p + margin)] / total
"""

import numpy as np

B, L, P, D = 32, 1024, 8, 256
NCORES = 8
BL = B // NCORES          # 4 splits per core
H = 2                     # d-halves (256 = 2*128)
C = 2                     # l-chunks (1024 = 2*512 psum-bank limit)
CH = 512
MARGIN = 0.01

_CACHED = {}


def _build_nc():
    import concourse.bass as bass
    import concourse.mybir as mybir
    import concourse.tile as tile
    from concourse import bacc

    f32 = mybir.dt.float32
    b16 = mybir.dt.bfloat16
    Alu = mybir.AluOpType
    Act = mybir.ActivationFunctionType

    nc = bacc.Bacc("TRN2")
    # x[b, p, h*1024 + l] = sent_T[b, 128h + p, l]  (host-transposed, bf16)
    x = nc.dram_tensor("x", [BL, 128, H * L], b16, kind="ExternalInput")
    # weight cols: block 4*(2b+h) has qhat[b, 128h:128h+128] at local col b;
    # block 32+4b has ones at local col b (ssq fold); cols 48:80 rows 0-3 are
    # the simrep indicator ind[b, 8b+j] = 1.
    wts = nc.dram_tensor("wts", [128, 80], b16, kind="ExternalInput")
    # aux[8b+j, 0:256] = sent[b, pos_idx[b,j], :]; aux[., 256:512] = qhat[b]
    aux = nc.dram_tensor("aux", [BL * P, 2 * D], b16, kind="ExternalInput")
    # out[8b+j] = (G[b,j], s_vec[b,j])
    out = nc.dram_tensor("out", [BL * P, 2], f32, kind="ExternalOutput")

    with tile.TileContext(nc) as tc:
        with (
            tc.tile_pool(name="singles", bufs=1) as singles,
            tc.tile_pool(name="xpool", bufs=4) as xpool,
            tc.tile_pool(name="sqpool", bufs=2) as sqpool,
            tc.tile_pool(name="pp", bufs=1, space="PSUM") as pp,
        ):
            # ---- ACT table warm: Sqrt forces the sqrt_and_others set, which
            # also holds Square/Relu/Copy -- one table load total.
            warm = singles.tile([1, 1], f32)
            nc.vector.memset(warm[:, :], 1.0)
            nc.scalar.activation(out=warm[0:1, :], in_=warm[0:1, :],
                                 func=Act.Sqrt)

            # ---- small loads on the SWDGE ring (parallel to x loads) ----
            wts_sb = singles.tile([128, 80], b16)
            nc.gpsimd.dma_start(out=wts_sb[:, :], in_=wts[:, :])
            aux_sb = singles.tile([BL * P, 2 * D], b16)
            nc.gpsimd.dma_start(out=aux_sb[:, :], in_=aux[:, :])

            # ---- x loads spread over the three DMA rings (the ACT
            # HWDGE ring is ~4x faster than the SP ring; SWDGE is third) ----
            xts = []
            x_eng = [nc.scalar, nc.scalar, nc.gpsimd, nc.sync]
            for b in range(BL):
                xt = xpool.tile([128, H * L], b16, tag=f"xt{b}")
                x_eng[b].dma_start(out=xt[:, :], in_=x[b, :, :])
                xts.append(xt)

            # ---- PE warm-up: keep the HAM busy during the DMA window so the
            # real matmuls run at 2.4 GHz instead of 1.2.
            junkw = singles.tile([128, 516], b16)
            nc.vector.memset(junkw[:, :], 0.125)
            pjunk = pp.tile([4, CH], f32, name="pjunk", tag="pjunk")
            for w in range(8):
                nc.tensor.matmul(pjunk[:, :], lhsT=junkw[:, 0:4],
                                 rhs=junkw[:, 4:516], start=True, stop=True)

            # ---- s_vec from host-gathered positives (under the x DMAs) ----
            dp = singles.tile([BL * P, 1], f32)
            sp = singles.tile([BL * P, 1], f32)
            junka = singles.tile([BL * P, D], b16)
            junkb = singles.tile([BL * P, D], b16)
            rtp = singles.tile([BL * P, 1], f32)
            svec = singles.tile([BL * P, 1], f32)
            ms = singles.tile([BL * P, 1], f32)
            outsb = singles.tile([BL * P, 2], f32)
            nc.vector.scalar_tensor_tensor(
                out=junka[:, :], in0=aux_sb[:, 0:D], scalar=1.0,
                in1=aux_sb[:, D:2 * D], op0=Alu.mult, op1=Alu.mult,
                accum_out=dp[:, :])
            nc.scalar.activation(out=junkb[:, :], in_=aux_sb[:, 0:D],
                                 func=Act.Square, accum_out=sp[:, :])
            nc.scalar.activation(out=rtp[:, :], in_=sp[:, :], func=Act.Sqrt)
            nc.vector.reciprocal(out=rtp[:, :], in_=rtp[:, :])
            nc.vector.tensor_mul(out=svec[:, :], in0=dp[:, :], in1=rtp[:, :])
            nc.vector.tensor_copy(out=outsb[:, 1:2], in_=svec[:, :])
            # ms = margin - s
            nc.scalar.activation(out=ms[:, :], in_=svec[:, :], func=Act.Copy,
                                 bias=float(MARGIN), scale=-1.0)

            # ---- heavy pass: dot rows prd[c], ssq rows prq[c] (psum) ----
            prd = [pp.tile([BL, CH], f32, name=f"prd{c}", tag=f"prd{c}")
                   for c in range(C)]
            prq = [pp.tile([BL, CH], f32, name=f"prq{c}", tag=f"prq{c}")
                   for c in range(C)]
            pss = [pp.tile([BL * P, CH], f32, name=f"ps{c}", tag=f"ps{c}")
                   for c in range(C)]
            nd = [0, 0]                   # dot MMs emitted per chunk
            nq = [0, 0]                   # fold MMs emitted per chunk
            ND_TOTAL = BL * H             # 8 dot MMs per chunk
            NQ_TOTAL = BL * H             # 8 fold MMs per chunk

            for b in [0, 1, 3, 2]:
                xt = xts[b]
                # dot: prd[c][b, :] += qhat[b,h] . x[b,h]
                for c in range(C):
                    for h in range(H):
                        nc.tensor.matmul(
                            prd[c][0:BL, :],
                            lhsT=wts_sb[:, 4 * (2 * b + h):4 * (2 * b + h) + 4],
                            rhs=xt[:, h * L + c * CH:h * L + c * CH + CH],
                            start=(nd[c] == 0),
                            stop=(nd[c] == ND_TOTAL - 1),
                        )
                        nd[c] += 1
                # squares (one big bf16 DVE op per split)
                sq = sqpool.tile([128, H * L], b16, tag="sq")
                nc.vector.tensor_mul(out=sq[:, :], in0=xt[:, :], in1=xt[:, :])
                for c in range(C):
                    for h in range(H):
                        nc.tensor.matmul(
                            prq[c][0:BL, :],
                            lhsT=wts_sb[:, 32 + 4 * b:32 + 4 * b + 4],
                            rhs=sq[:, h * L + c * CH:h * L + c * CH + CH],
                            start=(nq[c] == 0),
                            stop=(nq[c] == NQ_TOTAL - 1),
                        )
                        nq[c] += 1

            # ---- tail per chunk: sim rows -> broadcast -> relu+accum ----
            rt = singles.tile([BL, C, CH], f32)
            sim = singles.tile([BL, C, CH], b16)
            junkr = [singles.tile([BL * P, CH], b16, name=f"jr{c}", tag=f"jr{c}")
                     for c in range(C)]
            gg = singles.tile([BL * P, C], f32)
            for c in range(C):
                nc.scalar.activation(out=rt[:, c, :], in_=prq[c][0:BL, :],
                                     func=Act.Sqrt)
                nc.vector.reciprocal_approx_fast(out=rt[:, c, :],
                                                 in_=rt[:, c, :])
                nc.vector.tensor_mul(out=sim[:, c, :], in0=prd[c][0:BL, :],
                                     in1=rt[:, c, :])
                nc.tensor.matmul(pss[c][:, :], lhsT=wts_sb[0:BL, 48:80],
                                 rhs=sim[:, c, :], start=True, stop=True)
                nc.scalar.activation(out=junkr[c][:, :], in_=pss[c][:, :],
                                     func=Act.Relu, bias=ms[:, :],
                                     accum_out=gg[:, c:c + 1])
            nc.vector.tensor_add(out=outsb[:, 0:1], in0=gg[:, 0:1],
                                 in1=gg[:, 1:2])

            nc.sync.dma_start(out=out[:, :], in_=outsb[:, :])

    nc.finalize()
    return nc


def _get_nc():
    if "nc" not in _CACHED:
        _CACHED["nc"] = _build_nc()
    return _CACHED["nc"]


def _host_prep(sent, query, pos_idx):
    """Build per-core input maps (bf16, d-major x, packed weights)."""
    import ml_dtypes

    bf16 = ml_dtypes.bfloat16
    sent = np.asarray(sent, dtype=np.float32)
    query = np.asarray(query, dtype=np.float32)
    pos_idx = np.asarray(pos_idx).astype(np.int64)

    qn = np.linalg.norm(query, axis=-1, keepdims=True)
    qhat = (query / np.maximum(qn, 1e-12)).astype(bf16)

    sent16 = sent.astype(bf16)
    # [B, 128, H*L]: xt[b, p, h*L + l] = sent[b, l, 128h + p]
    xt = np.ascontiguousarray(
        sent16.transpose(0, 2, 1)             # [B, D, L]
        .reshape(B, H, 128, L)                # [B, h, p, l]
        .transpose(0, 2, 1, 3)                # [B, p, h, l]
        .reshape(B, 128, H * L))

    in_maps = []
    for core in range(NCORES):
        sl = slice(core * BL, (core + 1) * BL)
        q = qhat[sl]                          # [BL, D]
        wts = np.zeros((128, 80), dtype=bf16)
        for b in range(BL):
            for h in range(H):
                wts[:, 4 * (2 * b + h) + b] = q[b, 128 * h:128 * h + 128]
            wts[:, 32 + 4 * b + b] = 1.0
            wts[b, 48 + 8 * b:48 + 8 * b + 8] = 1.0
        aux = np.zeros((BL * P, 2 * D), dtype=bf16)
        for b in range(BL):
            gb = core * BL + b
            aux[8 * b:8 * b + 8, 0:D] = sent16[gb, pos_idx[gb], :]
            aux[8 * b:8 * b + 8, D:2 * D] = q[b]
        in_maps.append({
            "x": xt[sl],
            "wts": wts,
            "aux": np.ascontiguousarray(aux),
        })
    return in_maps, pos_idx


def _host_finish(results, pos_idx):
    """Combine per-core (G[b,j], s_vec[b,j]) into the scalar loss."""
    g = np.zeros((B, P), dtype=np.float64)
    s = np.zeros((B, P), dtype=np.float64)
    for core, res in enumerate(results):
        o = res["out"].reshape(BL, P, 2)
        g[core * BL:(core + 1) * BL] = o[:, :, 0]
        s[core * BL:(core + 1) * BL] = o[:, :, 1]

    loss = 0.0
    total = 0
    for b in range(B):
        _, first = np.unique(pos_idx[b], return_index=True)
        npos = len(first)
        total += npos * (L - npos)
        sb = s[b, first]
        loss += g[b, first].sum()
        loss -= np.maximum(sb[None, :] - sb[:, None] + MARGIN, 0.0).sum()
    return np.float32(loss / total)


def kernel(sent_embeddings, query_embeddings, pos_idx, splits=None, **_):
    import sys
    if "/opt/trn_rl_repo" not in sys.path:
        sys.path.insert(0, "/opt/trn_rl_repo")
    from concourse.bass_utils import run_bass_kernel_spmd

    in_maps, pos_idx = _host_prep(sent_embeddings, query_embeddings, pos_idx)
    nc = _get_nc()
    res = run_bass_kernel_spmd(nc, in_maps, core_ids=list(range(NCORES)))
    _CACHED["last_result"] = res
    return _host_finish(res.results, pos_idx)


if __name__ == "__main__":
    rng = np.random.default_rng(0)
    sent = rng.standard_normal((B, L, D), dtype=np.float32)
    query = rng.standard_normal((B, D), dtype=np.float32)
    pidx = np.stack([rng.choice(L, P, replace=False) for _ in range(B)])
    print(kernel(sent, query, pidx, L))


# revision 7
# speedup vs baseline: 1.3407x; 1.0354x over previous
"""Trainium2 Bass kernel for a contrastive hinge loss.

Problem (B=32 splits, L=1024 candidates/split, P=8 positives/split, D=256):
    e = l2norm(sent), q = l2norm(query)
    sim[b,l] = e[b,l] . q[b]
    loss = sum_{b, p in pos_b, j in neg_b} relu(sim[b,j] - sim[b,p] + margin) / total

Strategy (data-parallel over B across 8 cores, 4 splits per core), v2:
  Inputs are downcast to bf16 on the host (tolerance is 2e-2; bf16 keeps the
  loss within ~1e-3) which halves HBM traffic -- the roofline for this
  memory-regime problem -- and x is sent pre-transposed to d-major layout
  [d, l] so the 256-long dot reductions run on the PE (128 MACs/col/cycle)
  instead of element-serial DVE/ACT accumulation:
    - dot rows:  prd[b, l] = sum_d q[b,d] x[b,d,l]   (PE, q as 4-col weights)
    - ssq rows:  prq[b, l] = sum_d x[b,d,l]^2        (DVE square, PE ones-fold;
                                                      gpsimd pre-folds the two
                                                      d-halves for 2 splits)
    - sim = dot * rsqrt(ssq) on [4, 512] rows (ACT sqrt, DVE approx-recip, mult)
    - one tiny PE matmul broadcasts sim rows to 32 (b,j) partitions, then a
      SINGLE ACT Relu with per-partition bias (margin - s_bj) and accumulator
      computes G[b,j] = sum_l relu(sim_l - s_bj + margin) per chunk.
    - s_vec[b,j] comes from host-gathered positive embeddings xpos [32, 256]
      (candidate-major, partition = (b,j)): STT dot + ACT square-accum +
      sqrt/recip/mult. This replaces the one-hot matmul gather entirely.
  Host: normalizes queries, gathers xpos, packs weights; finishes with the
  same dedup + pos-pos hinge correction as before:
    loss = [sum G[b,j] - sum_{p,q in pos_b} relu(s_q - s_p + margin)] / total
"""

import numpy as np

B, L, P, D = 32, 1024, 8, 256
NCORES = 8
BL = B // NCORES          # 4 splits per core
H = 2                     # d-halves (256 = 2*128)
C = 2                     # l-chunks (1024 = 2*512 psum-bank limit)
CH = 512
MARGIN = 0.01

_CACHED = {}


def _build_nc():
    import concourse.bass as bass
    import concourse.mybir as mybir
    import concourse.tile as tile
    from concourse import bacc

    f32 = mybir.dt.float32
    b16 = mybir.dt.bfloat16
    Alu = mybir.AluOpType
    Act = mybir.ActivationFunctionType

    nc = bacc.Bacc("TRN2")
    # x[b, p, h*1024 + l] = sent_T[b, 128h + p, l]  (host-transposed, bf16).
    # Splits 0+1 ride one tensor so each partition line is 8 KB contiguous --
    # the HWDGE rings are descriptor-rate-limited, so bigger lines = more B/s.
    xp = nc.dram_tensor("xp", [128, 2 * H * L], b16, kind="ExternalInput")
    x2 = nc.dram_tensor("x2", [128, H * L], b16, kind="ExternalInput")
    x3 = nc.dram_tensor("x3", [128, H * L], b16, kind="ExternalInput")
    # weight cols: block 4*(2b+h) has qhat[b, 128h:128h+128] at local col b;
    # block 32+4b has ones at local col b (ssq fold); cols 48:80 rows 0-3 are
    # the simrep indicator ind[b, 8b+j] = 1.
    wts = nc.dram_tensor("wts", [128, 80], b16, kind="ExternalInput")
    # aux[8b+j, 0:256] = sent[b, pos_idx[b,j], :]; aux[., 256:512] = qhat[b]
    aux = nc.dram_tensor("aux", [BL * P, 2 * D], b16, kind="ExternalInput")
    # out[8b+j] = (G[b,j], s_vec[b,j])
    out = nc.dram_tensor("out", [BL * P, 2], f32, kind="ExternalOutput")

    with tile.TileContext(nc) as tc:
        with (
            tc.tile_pool(name="singles", bufs=1) as singles,
            tc.tile_pool(name="xpool", bufs=4) as xpool,
            tc.tile_pool(name="sqpool", bufs=4) as sqpool,
            tc.tile_pool(name="pp", bufs=1, space="PSUM") as pp,
        ):
            # ---- ACT table warm: Sqrt forces the sqrt_and_others set, which
            # also holds Square/Relu/Copy -- one table load total.
            warm = singles.tile([1, 1], f32)
            nc.vector.memset(warm[:, :], 1.0)
            nc.scalar.activation(out=warm[0:1, :], in_=warm[0:1, :],
                                 func=Act.Sqrt)

            # ---- small loads on the SWDGE ring (parallel to x loads) ----
            wts_sb = singles.tile([128, 80], b16)
            nc.gpsimd.dma_start(out=wts_sb[:, :], in_=wts[:, :])
            aux_sb = singles.tile([BL * P, 2 * D], b16)
            nc.gpsimd.dma_start(out=aux_sb[:, :], in_=aux[:, :])

            # ---- x loads: ACT HWDGE ring is ~4x faster than the SP
            # ring, SWDGE in between; SP only carries the output store. ----
            xp_sb = xpool.tile([128, 2 * H * L], b16, tag="xp")
            nc.scalar.dma_start(out=xp_sb[:, :], in_=xp[:, :])
            xt3 = xpool.tile([128, H * L], b16, tag="xt3")
            nc.scalar.dma_start(out=xt3[:, :], in_=x3[:, :])
            xt2 = xpool.tile([128, H * L], b16, tag="xt2")
            nc.gpsimd.dma_start(out=xt2[:, :], in_=x2[:, :])
            xts = [xp_sb[:, 0:H * L], xp_sb[:, H * L:2 * H * L], xt2, xt3]

            # ---- PE warm-up: keep the HAM busy during the DMA window so the
            # real matmuls run at 2.4 GHz instead of 1.2.
            junkw = singles.tile([128, 516], b16)
            nc.vector.memset(junkw[:, :], 0.125)
            pjunk = pp.tile([4, CH], f32, name="pjunk", tag="pjunk")
            for w in range(8):
                nc.tensor.matmul(pjunk[:, :], lhsT=junkw[:, 0:4],
                                 rhs=junkw[:, 4:516], start=True, stop=True)

            # ---- s_vec from host-gathered positives (under the x DMAs) ----
            dp = singles.tile([BL * P, 1], f32)
            sp = singles.tile([BL * P, 1], f32)
            junka = singles.tile([BL * P, D], b16)
            junkb = singles.tile([BL * P, D], b16)
            rtp = singles.tile([BL * P, 1], f32)
            svec = singles.tile([BL * P, 1], f32)
            ms = singles.tile([BL * P, 1], f32)
            outsb = singles.tile([BL * P, 2], f32)
            nc.vector.scalar_tensor_tensor(
                out=junka[:, :], in0=aux_sb[:, 0:D], scalar=1.0,
                in1=aux_sb[:, D:2 * D], op0=Alu.mult, op1=Alu.mult,
                accum_out=dp[:, :])
            nc.scalar.activation(out=junkb[:, :], in_=aux_sb[:, 0:D],
                                 func=Act.Square, accum_out=sp[:, :])
            nc.scalar.activation(out=rtp[:, :], in_=sp[:, :], func=Act.Sqrt)
            nc.vector.reciprocal(out=rtp[:, :], in_=rtp[:, :])
            nc.vector.tensor_mul(out=svec[:, :], in0=dp[:, :], in1=rtp[:, :])
            nc.vector.tensor_copy(out=outsb[:, 1:2], in_=svec[:, :])
            # ms = margin - s
            nc.scalar.activation(out=ms[:, :], in_=svec[:, :], func=Act.Copy,
                                 bias=float(MARGIN), scale=-1.0)

            # ---- heavy pass: dot rows prd[c], ssq rows prq[c] (psum) ----
            prd = [pp.tile([BL, CH], f32, name=f"prd{c}", tag=f"prd{c}")
                   for c in range(C)]
            prq = [pp.tile([BL, CH], f32, name=f"prq{c}", tag=f"prq{c}")
                   for c in range(C)]
            pss = [pp.tile([BL * P, CH], f32, name=f"ps{c}", tag=f"ps{c}")
                   for c in range(C)]
            nd = [0, 0]                   # dot MMs emitted per chunk
            nq = [0, 0]                   # fold MMs emitted per chunk
            ND_TOTAL = BL * H             # 8 dot MMs per chunk
            NQ_TOTAL = BL * H             # 8 fold MMs per chunk

            for b in range(BL):
                xt = xts[b]
                # dot: prd[c][b, :] += qhat[b,h] . x[b,h]
                # (h-major so consecutive MMs share lhsT -> LDWEIGHTS dedup)
                for h in range(H):
                    for c in range(C):
                        nc.tensor.matmul(
                            prd[c][0:BL, :],
                            lhsT=wts_sb[:, 4 * (2 * b + h):4 * (2 * b + h) + 4],
                            rhs=xt[:, h * L + c * CH:h * L + c * CH + CH],
                            start=(nd[c] == 0),
                            stop=(nd[c] == ND_TOTAL - 1),
                        )
                        nd[c] += 1
                # squares (one big bf16 DVE op per split)
                sq = sqpool.tile([128, H * L], b16, tag="sq")
                nc.vector.tensor_mul(out=sq[:, :], in0=xt[:, :], in1=xt[:, :])
                for c in range(C):
                    for h in range(H):
                        nc.tensor.matmul(
                            prq[c][0:BL, :],
                            lhsT=wts_sb[:, 32 + 4 * b:32 + 4 * b + 4],
                            rhs=sq[:, h * L + c * CH:h * L + c * CH + CH],
                            start=(nq[c] == 0),
                            stop=(nq[c] == NQ_TOTAL - 1),
                        )
                        nq[c] += 1

            # ---- tail per chunk: sim rows -> broadcast -> relu+accum ----
            rt = singles.tile([BL, C, CH], f32)
            sim = singles.tile([BL, C, CH], b16)
            junkr = [singles.tile([BL * P, CH], b16, name=f"jr{c}", tag=f"jr{c}")
                     for c in range(C)]
            gg = singles.tile([BL * P, C], f32)
            for c in range(C):
                nc.scalar.activation(out=rt[:, c, :], in_=prq[c][0:BL, :],
                                     func=Act.Sqrt)
                nc.vector.reciprocal_approx_fast(out=rt[:, c, :],
                                                 in_=rt[:, c, :])
                nc.vector.tensor_mul(out=sim[:, c, :], in0=prd[c][0:BL, :],
                                     in1=rt[:, c, :])
                nc.tensor.matmul(pss[c][:, :], lhsT=wts_sb[0:BL, 48:80],
                                 rhs=sim[:, c, :], start=True, stop=True)
                nc.scalar.activation(out=junkr[c][:, :], in_=pss[c][:, :],
                                     func=Act.Relu, bias=ms[:, :],
                                     accum_out=gg[:, c:c + 1])
            nc.vector.tensor_add(out=outsb[:, 0:1], in0=gg[:, 0:1],
                                 in1=gg[:, 1:2])

            nc.sync.dma_start(out=out[:, :], in_=outsb[:, :])

    nc.finalize()
    return nc


def _get_nc():
    if "nc" not in _CACHED:
        _CACHED["nc"] = _build_nc()
    return _CACHED["nc"]


def _host_prep(sent, query, pos_idx):
    """Build per-core input maps (bf16, d-major x, packed weights)."""
    import ml_dtypes

    bf16 = ml_dtypes.bfloat16
    sent = np.asarray(sent, dtype=np.float32)
    query = np.asarray(query, dtype=np.float32)
    pos_idx = np.asarray(pos_idx).astype(np.int64)

    qn = np.linalg.norm(query, axis=-1, keepdims=True)
    qhat = (query / np.maximum(qn, 1e-12)).astype(bf16)

    sent16 = sent.astype(bf16)
    # [B, 128, H*L]: xt[b, p, h*L + l] = sent[b, l, 128h + p]
    xt = np.ascontiguousarray(
        sent16.transpose(0, 2, 1)             # [B, D, L]
        .reshape(B, H, 128, L)                # [B, h, p, l]
        .transpose(0, 2, 1, 3)                # [B, p, h, l]
        .reshape(B, 128, H * L))

    in_maps = []
    for core in range(NCORES):
        sl = slice(core * BL, (core + 1) * BL)
        q = qhat[sl]                          # [BL, D]
        wts = np.zeros((128, 80), dtype=bf16)
        for b in range(BL):
            for h in range(H):
                wts[:, 4 * (2 * b + h) + b] = q[b, 128 * h:128 * h + 128]
            wts[:, 32 + 4 * b + b] = 1.0
            wts[b, 48 + 8 * b:48 + 8 * b + 8] = 1.0
        aux = np.zeros((BL * P, 2 * D), dtype=bf16)
        for b in range(BL):
            gb = core * BL + b
            aux[8 * b:8 * b + 8, 0:D] = sent16[gb, pos_idx[gb], :]
            aux[8 * b:8 * b + 8, D:2 * D] = q[b]
        xc = xt[sl]
        in_maps.append({
            "xp": np.ascontiguousarray(
                np.concatenate([xc[0], xc[1]], axis=1)),
            "x2": xc[2],
            "x3": xc[3],
            "wts": wts,
            "aux": np.ascontiguousarray(aux),
        })
    return in_maps, pos_idx


def _host_finish(results, pos_idx):
    """Combine per-core (G[b,j], s_vec[b,j]) into the scalar loss."""
    g = np.zeros((B, P), dtype=np.float64)
    s = np.zeros((B, P), dtype=np.float64)
    for core, res in enumerate(results):
        o = res["out"].reshape(BL, P, 2)
        g[core * BL:(core + 1) * BL] = o[:, :, 0]
        s[core * BL:(core + 1) * BL] = o[:, :, 1]

    loss = 0.0
    total = 0
    for b in range(B):
        _, first = np.unique(pos_idx[b], return_index=True)
        npos = len(first)
        total += npos * (L - npos)
        sb = s[b, first]
        loss += g[b, first].sum()
        loss -= np.maximum(sb[None, :] - sb[:, None] + MARGIN, 0.0).sum()
    return np.float32(loss / total)


def kernel(sent_embeddings, query_embeddings, pos_idx, splits=None, **_):
    import sys
    if "/opt/trn_rl_repo" not in sys.path:
        sys.path.insert(0, "/opt/trn_rl_repo")
    from concourse.bass_utils import run_bass_kernel_spmd

    in_maps, pos_idx = _host_prep(sent_embeddings, query_embeddings, pos_idx)
    nc = _get_nc()
    res = run_bass_kernel_spmd(nc, in_maps, core_ids=list(range(NCORES)))
    _CACHED["last_result"] = res
    return _host_finish(res.results, pos_idx)


if __name__ == "__main__":
    rng = np.random.default_rng(0)
    sent = rng.standard_normal((B, L, D), dtype=np.float32)
    query = rng.standard_normal((B, D), dtype=np.float32)
    pidx = np.stack([rng.choice(L, P, replace=False) for _ in range(B)])
    print(kernel(sent, query, pidx, L))


# revision 8
# speedup vs baseline: 1.3906x; 1.0372x over previous
"""Trainium2 Bass kernel for a contrastive hinge loss.

Problem (B=32 splits, L=1024 candidates/split, P=8 positives/split, D=256):
    e = l2norm(sent), q = l2norm(query)
    sim[b,l] = e[b,l] . q[b]
    loss = sum_{b, p in pos_b, j in neg_b} relu(sim[b,j] - sim[b,p] + margin) / total

Strategy (data-parallel over B across 8 cores, 4 splits per core), v5:
  bf16 inputs (tolerance 2e-2; bf16 keeps the loss within ~2e-5) halve the
  HBM traffic, and x is host-transposed to d-major [d, l] so the dot
  reductions run on the PE.  All dot/ssq results live in ONE [8, 512] psum
  row tile each, partition (4c + b) = l-chunk c of split b, so the whole
  normalization + hinge tail is a single chain:
    prd[4c+b, :] = sum_d q[b,d] x[b,d,l]     (PE, q as 8-col weights)
    prq[4c+b, :] = sum_d x[b,d,l]^2          (squares on DVE for 2 splits and
                                              ACT for 2, PE ones-fold)
    sim = prd * rsqrt(prq)                   (ACT sqrt, DVE approx-recip, mult)
    pss[64, 512] = Ind^T @ sim               (one PE matmul: broadcast to all
                                              (c, b, j) partitions)
    G2[(c,b,j)] = sum_l relu(pss + (margin - s_bj))   (ONE ACT Relu with
                                              per-partition bias + accumulator)
  G[b,j] = G2[0,b,j] + G2[1,b,j] summed on the host.
  s_vec[b,j] comes from host-gathered positive embeddings (aux), computed on
  partitions (c,b,j) directly: STT dot + ACT square-accum + sqrt/recip/mult.
  DMA: the ACT HWDGE ring is ~3-4x faster than the SP ring, so x rides ACT
  (splits 0+1 packed as one tensor for 8 KB descriptor lines, split 3 second)
  plus SWDGE (split 2); wts/aux go on the idle SP ring; out returns on ACT.
  Host finish: dedup positives + subtract pos-pos hinge terms, divide by total.
"""

import numpy as np

B, L, P, D = 32, 1024, 8, 256
NCORES = 8
BL = B // NCORES          # 4 splits per core
H = 2                     # d-halves (256 = 2*128)
C = 2                     # l-chunks (1024 = 2*512 psum-bank limit)
CH = 512
MARGIN = 0.01

# wts column layout (all bf16):
#   dot block for (b,h): 12 cols at WD + 12*(2b+h); q sits at local col 4+b,
#     so slice [4-4c : 12-4c] puts q at col (4c+b) of an 8-wide lhsT.
#   fold block for b: 12 cols at WF + 12*b; ones at local col 4+b, same trick.
#   ind block: 64 cols at WI; wts[4c+b, WI + 32c + 8b + j] = 1.
WD = 0
WF = WD + 12 * BL * H     # 96
WI = WF + 12 * BL         # 144
WTOT = WI + 2 * BL * P    # 208

_CACHED = {}


def _build_nc():
    import concourse.bass as bass
    import concourse.mybir as mybir
    import concourse.tile as tile
    from concourse import bacc

    f32 = mybir.dt.float32
    b16 = mybir.dt.bfloat16
    Alu = mybir.AluOpType
    Act = mybir.ActivationFunctionType

    nc = bacc.Bacc("TRN2")
    # x[b][p, h*1024 + l] = sent_T[b, 128h + p, l]  (host-transposed, bf16).
    # Splits 0+1 ride one tensor so each partition line is 8 KB contiguous --
    # the HWDGE rings are descriptor-rate-limited, so bigger lines = more B/s.
    xp = nc.dram_tensor("xp", [128, 2 * H * L], b16, kind="ExternalInput")
    x2 = nc.dram_tensor("x2", [128, H * L], b16, kind="ExternalInput")
    x3 = nc.dram_tensor("x3", [128, H * L], b16, kind="ExternalInput")
    wts = nc.dram_tensor("wts", [128, WTOT], b16, kind="ExternalInput")
    # aux[32c + 8b + j, 0:256] = sent[b, pos_idx[b,j], :]; [., 256:512] = qhat[b]
    aux = nc.dram_tensor("aux", [2 * BL * P, 2 * D], b16, kind="ExternalInput")
    # out[32c + 8b + j] = (G2[c,b,j], s_vec[b,j])
    out = nc.dram_tensor("out", [2 * BL * P, 2], f32, kind="ExternalOutput")

    NP = 2 * BL * P       # 64 (c,b,j) partitions

    with tile.TileContext(nc) as tc:
        with (
            tc.tile_pool(name="singles", bufs=1) as singles,
            tc.tile_pool(name="xpool", bufs=4) as xpool,
            tc.tile_pool(name="sqpool", bufs=4) as sqpool,
            tc.tile_pool(name="pp", bufs=1, space="PSUM") as pp,
        ):
            # ---- x loads first on their rings ----
            xp_sb = xpool.tile([128, 2 * H * L], b16, tag="xp")
            nc.scalar.dma_start(out=xp_sb[:, :], in_=xp[:, :])
            xt3 = xpool.tile([128, H * L], b16, tag="xt3")
            nc.scalar.dma_start(out=xt3[:, :], in_=x3[:, :])
            xt2 = xpool.tile([128, H * L], b16, tag="xt2")
            nc.gpsimd.dma_start(out=xt2[:, :], in_=x2[:, :])
            xts = [xp_sb[:, 0:H * L], xp_sb[:, H * L:2 * H * L], xt2, xt3]

            # small loads on the otherwise idle SP ring
            wts_sb = singles.tile([128, WTOT], b16)
            nc.sync.dma_start(out=wts_sb[:, :], in_=wts[:, :])
            aux_sb = singles.tile([NP, 2 * D], b16)
            nc.sync.dma_start(out=aux_sb[:, :], in_=aux[:, :])

            # ---- ACT table warm (sqrt_and_others: Sqrt/Square/Relu/Copy) ----
            warm = singles.tile([1, 1], f32)
            nc.vector.memset(warm[:, :], 1.0)
            nc.scalar.activation(out=warm[0:1, :], in_=warm[0:1, :],
                                 func=Act.Sqrt)

            # ---- PE warm-up: keep the HAM busy through the DMA window so
            # real matmuls run at 2.4 GHz instead of 1.2.
            junkw = singles.tile([128, 516], b16)
            nc.vector.memset(junkw[:, :], 0.125)
            pjunk = pp.tile([4, CH], f32, name="pjunk", tag="pjunk")
            for w in range(10):
                nc.tensor.matmul(pjunk[:, :], lhsT=junkw[:, 0:4],
                                 rhs=junkw[:, 4:516], start=True, stop=True)

            # ---- heavy pass: dot rows prd, ssq rows prq at (4c+b) ----
            prd = pp.tile([2 * BL, CH], f32, name="prd", tag="prd")
            prq = pp.tile([2 * BL, CH], f32, name="prq", tag="prq")
            pss = pp.tile([NP, CH], f32, name="pss", tag="pss")

            nd = 0
            nq = 0
            NTOT = BL * H * C     # 16 MMs in each of the two groups

            for b in [0, 1, 3, 2]:
                xt = xts[b]
                # dot: prd[4c+b, :] += qhat[b,h] . x[b,h,chunk c]
                for h in range(H):
                    blk = WD + 12 * (2 * b + h)
                    for c in range(C):
                        nc.tensor.matmul(
                            prd[:, :],
                            lhsT=wts_sb[:, blk + 4 - 4 * c:blk + 12 - 4 * c],
                            rhs=xt[:, h * L + c * CH:h * L + c * CH + CH],
                            start=(nd == 0),
                            stop=(nd == NTOT - 1),
                        )
                        nd += 1
                # squares: one big bf16 op per split, alternating DVE/ACT so
                # the four don't serialize on one engine
                sq = sqpool.tile([128, H * L], b16, tag="sq")
                if b in (0, 2):
                    nc.vector.tensor_mul(out=sq[:, :], in0=xt[:, :],
                                         in1=xt[:, :])
                else:
                    nc.scalar.activation(out=sq[:, :], in_=xt[:, :],
                                         func=Act.Square)
                fblk = WF + 12 * b
                for c in range(C):
                    for h in range(H):
                        nc.tensor.matmul(
                            prq[:, :],
                            lhsT=wts_sb[:, fblk + 4 - 4 * c:fblk + 12 - 4 * c],
                            rhs=sq[:, h * L + c * CH:h * L + c * CH + CH],
                            start=(nq == 0),
                            stop=(nq == NTOT - 1),
                        )
                        nq += 1

            # ---- s_vec on (c,b,j) partitions (emitted late: only needed as
            # the relu bias; aux also arrives late on the SP ring) ----
            dp = singles.tile([NP, 1], f32)
            sp = singles.tile([NP, 1], f32)
            junka = singles.tile([NP, D], b16)
            junkb = singles.tile([NP, D], b16)
            rtp = singles.tile([NP, 1], f32)
            svec = singles.tile([NP, 1], f32)
            ms = singles.tile([NP, 1], f32)
            outsb = singles.tile([NP, 2], f32)
            nc.vector.scalar_tensor_tensor(
                out=junka[:, :], in0=aux_sb[:, 0:D], scalar=1.0,
                in1=aux_sb[:, D:2 * D], op0=Alu.mult, op1=Alu.mult,
                accum_out=dp[:, :])
            nc.scalar.activation(out=junkb[:, :], in_=aux_sb[:, 0:D],
                                 func=Act.Square, accum_out=sp[:, :])
            nc.scalar.activation(out=rtp[:, :], in_=sp[:, :], func=Act.Sqrt)
            nc.vector.reciprocal(out=rtp[:, :], in_=rtp[:, :])
            nc.vector.tensor_mul(out=svec[:, :], in0=dp[:, :], in1=rtp[:, :])
            nc.vector.tensor_copy(out=outsb[:, 1:2], in_=svec[:, :])
            # ms = margin - s
            nc.scalar.activation(out=ms[:, :], in_=svec[:, :], func=Act.Copy,
                                 bias=float(MARGIN), scale=-1.0)

            # ---- single tail: sim rows -> broadcast -> relu+accum ----
            rt = singles.tile([2 * BL, CH], f32)
            sim = singles.tile([2 * BL, CH], b16)
            junkr = singles.tile([NP, CH], b16)
            nc.scalar.activation(out=rt[:, :], in_=prq[:, :], func=Act.Sqrt)
            nc.vector.reciprocal_approx_fast(out=rt[:, :], in_=rt[:, :])
            nc.vector.tensor_mul(out=sim[:, :], in0=prd[:, :], in1=rt[:, :])
            nc.tensor.matmul(pss[:, :], lhsT=wts_sb[0:2 * BL, WI:WI + NP],
                             rhs=sim[:, :], start=True, stop=True)
            nc.scalar.activation(out=junkr[:, :], in_=pss[:, :],
                                 func=Act.Relu, bias=ms[:, :],
                                 accum_out=outsb[:, 0:1])

            nc.scalar.dma_start(out=out[:, :], in_=outsb[:, :])

    nc.finalize()
    return nc


def _get_nc():
    if "nc" not in _CACHED:
        _CACHED["nc"] = _build_nc()
    return _CACHED["nc"]


def _host_prep(sent, query, pos_idx):
    """Build per-core input maps (bf16, d-major x, packed weights)."""
    import ml_dtypes

    bf16 = ml_dtypes.bfloat16
    sent = np.asarray(sent, dtype=np.float32)
    query = np.asarray(query, dtype=np.float32)
    pos_idx = np.asarray(pos_idx).astype(np.int64)

    qn = np.linalg.norm(query, axis=-1, keepdims=True)
    qhat = (query / np.maximum(qn, 1e-12)).astype(bf16)

    sent16 = sent.astype(bf16)
    # [B, 128, H*L]: xt[b, p, h*L + l] = sent[b, l, 128h + p]
    xt = np.ascontiguousarray(
        sent16.transpose(0, 2, 1)             # [B, D, L]
        .reshape(B, H, 128, L)                # [B, h, p, l]
        .transpose(0, 2, 1, 3)                # [B, p, h, l]
        .reshape(B, 128, H * L))

    in_maps = []
    for core in range(NCORES):
        sl = slice(core * BL, (core + 1) * BL)
        q = qhat[sl]                          # [BL, D]
        wts = np.zeros((128, WTOT), dtype=bf16)
        for b in range(BL):
            for h in range(H):
                wts[:, WD + 12 * (2 * b + h) + 4 + b] = \
                    q[b, 128 * h:128 * h + 128]
            wts[:, WF + 12 * b + 4 + b] = 1.0
            for c in range(C):
                for j in range(P):
                    wts[4 * c + b, WI + 32 * c + 8 * b + j] = 1.0
        aux = np.zeros((2 * BL * P, 2 * D), dtype=bf16)
        for c in range(C):
            for b in range(BL):
                gb = core * BL + b
                r = slice(32 * c + 8 * b, 32 * c + 8 * b + 8)
                aux[r, 0:D] = sent16[gb, pos_idx[gb], :]
                aux[r, D:2 * D] = q[b]
        xc = xt[sl]
        in_maps.append({
            "xp": np.ascontiguousarray(
                np.concatenate([xc[0], xc[1]], axis=1)),
            "x2": xc[2],
            "x3": xc[3],
            "wts": wts,
            "aux": np.ascontiguousarray(aux),
        })
    return in_maps, pos_idx


def _host_finish(results, pos_idx):
    """Combine per-core (G2[c,b,j], s_vec[b,j]) into the scalar loss."""
    g = np.zeros((B, P), dtype=np.float64)
    s = np.zeros((B, P), dtype=np.float64)
    for core, res in enumerate(results):
        o = res["out"].reshape(C, BL, P, 2)
        g[core * BL:(core + 1) * BL] = o[:, :, :, 0].sum(axis=0)
        s[core * BL:(core + 1) * BL] = o[0, :, :, 1]

    loss = 0.0
    total = 0
    for b in range(B):
        _, first = np.unique(pos_idx[b], return_index=True)
        npos = len(first)
        total += npos * (L - npos)
        sb = s[b, first]
        loss += g[b, first].sum()
        loss -= np.maximum(sb[None, :] - sb[:, None] + MARGIN, 0.0).sum()
    return np.float32(loss / total)


def kernel(sent_embeddings, query_embeddings, pos_idx, splits=None, **_):
    import sys
    if "/opt/trn_rl_repo" not in sys.path:
        sys.path.insert(0, "/opt/trn_rl_repo")
    from concourse.bass_utils import run_bass_kernel_spmd

    in_maps, pos_idx = _host_prep(sent_embeddings, query_embeddings, pos_idx)
    nc = _get_nc()
    res = run_bass_kernel_spmd(nc, in_maps, core_ids=list(range(NCORES)))
    _CACHED["last_result"] = res
    return _host_finish(res.results, pos_idx)


if __name__ == "__main__":
    rng = np.random.default_rng(0)
    sent = rng.standard_normal((B, L, D), dtype=np.float32)
    query = rng.standard_normal((B, D), dtype=np.float32)
    pidx = np.stack([rng.choice(L, P, replace=False) for _ in range(B)])
    print(kernel(sent, query, pidx, L))
